# revision 29
# baseline (speedup 1.0000x reference)
"""Character-LSTM Trainium2 kernel V3 (8 NeuronCores, SPMD data-parallel).

Strategy
--------
Words sorted by descending length are dealt round-robin across 8 cores so the
per-step active-column count A[t] is core-uniform; words live as columns of
transposed state tiles.

The first two LSTM steps are resolved on the host from input-independent
weight tables: step 0's state (c1, h1) is a function of the first char only
(V-entry table), and step 1's state (c2, h2) a function of the first char
pair (V^2-entry table). Words of length <= 2 never reach the device; longer
words start on-device at t=2 with (c2, h2) DMA'd in as initial state (c2 in
f16, h2 in fp8). This removes the two widest recurrence steps (~15% of
column-steps) from the device program, whose throughput is bound by the
Activation engine (10 table-lookup elems per column-step).

Per step, gates g[4H x cols] are computed on the PE in Double-FP8: the x-part
uses a host-built fp8 one-hot of the char against a hi/lo pair of fp8 embproj
tables (emb@W_ih.T + bias, value + e4m3 residual -> ~f16 accuracy), the
h-part uses fp8 W_hh x fp8 h; each DoubleRow matmul contracts K=256 in one
pass. Activations run on ACT (one instruction per gate over both 128-row
chunks via a 2D access pattern) with the x16 table scaling folded into the
ACT scale. The cell update runs on DVE in f16; h is stored fp8 for the next
step's matmuls (on Pool, freeing DVE), and in fp32 for columns finishing at
this step. PE sweeps that do not depend on h are emitted before the h sweeps
to keep the recurrence critical path short, and finished columns stream to
HBM per step.
"""

import sys

if "/opt/trn_rl_repo" not in sys.path:
    sys.path.insert(0, "/opt/trn_rl_repo")

import hashlib

import numpy as np
import ml_dtypes

import concourse.bass as bass
import concourse.tile as tile
from concourse import bacc, mybir
from concourse.bass_utils import run_bass_kernel_spmd

E4 = ml_dtypes.float8_e4m3fn
NCORES = 8
B, S, W, E, H, V = 64, 256, 24, 128, 256, 256
HOST_STEPS = 2  # steps resolved on host via vocab/pair tables
WD = W - HOST_STEPS  # device steps
GATE_FUNCS = ["Sigmoid", "Sigmoid", "Tanh", "Sigmoid"]  # i, f, g, o
GORDER = (0, 2, 1, 3)  # emission order: i, g, f, o
SC = 16.0  # fp8 table scale; ACT applies 1/SC
CH = 512  # column chunk (PSUM: 4 gate tags x [128, 2*CH] fp32 = 8 banks)
MMW = 512  # DoubleRow matmul window (one PSUM bank of output)

_PROGRAM_CACHE: dict = {}
_TABLE_CACHE: dict = {}


def _plan(lens: np.ndarray, round_to: int = 1):
    """Column counts per device step, uniform across cores.

    lens here are DEVICE lens (len-2, in [1, WD]); words with len<=2 are
    excluded before calling.
    """
    wL = np.bincount(lens, minlength=WD + 1)
    colsL = np.zeros(WD + 1, np.int64)
    cum = 0
    for L in range(WD, 0, -1):
        need = -(-int(wL[L]) // NCORES)
        newcum = -(-(cum + need) // round_to) * round_to
        colsL[L] = newcum - cum
        cum = newcum
    C = max(cum, 16)
    A = [int(colsL[t + 1 :].sum()) for t in range(WD)]
    return colsL, C, A


def _assign(lens, ids_all, colsL, C):
    """Deal device words into (core, column) slots, longest first."""
    order = np.argsort(-lens, kind="stable")
    wL = np.bincount(lens, minlength=WD + 1)
    colmap = np.full((NCORES, C), -1, np.int64)
    pos = 0
    s = 0
    for L in range(WD, 0, -1):
        cnt = int(wL[L])
        if cnt:
            sel = order[pos : pos + cnt]
            pos += cnt
            k = np.arange(cnt) % NCORES
            j = s + np.arange(cnt) // NCORES
            colmap[k, j] = ids_all[sel]
        s += int(colsL[L])
    return colmap


def _pl(t, stride, lo, w):
    """AP over tile t: [128 part, 2 planes (stride), w cols] at col offset lo."""
    return bass.AP(
        tensor=t.tensor, offset=t.offset + lo, ap=[t.ap[0], [stride, 2], [1, w]]
    )


def _build_program(C: int, A: list[int], reps: int = 1, ch: int = CH, mmw: int = MMW,
                   hsplit: int = 1, h8_pool: bool = False, pair_if: bool = True,
                   h8win: bool = False, sigsplit: bool = False,
                   tansplit: bool = False, use_epl: bool = True,
                   hf_pool: bool = True):
    key = (C, tuple(A), reps, ch, mmw, hsplit, h8_pool, pair_if, h8win, sigsplit,
           tansplit, use_epl, hf_pool)
    if key in _PROGRAM_CACHE:
        return _PROGRAM_CACHE[key]

    dt = mybir.dt
    AF = mybir.ActivationFunctionType
    DR = mybir.MatmulPerfMode.DoubleRow
    nc = bacc.Bacc("TRN2", target_bir_lowering=False, debug=False, num_devices=NCORES)

    oh_d = nc.dram_tensor("oh", [WD, 128, 2 * C], dt.float8e4, kind="ExternalInput")
    eph_d = nc.dram_tensor("eph", [128, 2 * 4 * H], dt.float8e4, kind="ExternalInput")
    epl_d = nc.dram_tensor("epl", [128, 2 * 4 * H], dt.float8e4, kind="ExternalInput")
    whh_d = nc.dram_tensor("whh", [128, 2 * 4 * H], dt.float8e4, kind="ExternalInput")
    c0_d = nc.dram_tensor("c0", [128, 2 * C], dt.float16, kind="ExternalInput")
    h0_d = nc.dram_tensor("h0", [128, 2 * C], dt.float8e4, kind="ExternalInput")
    out_d = nc.dram_tensor("out", [2, 128, C], dt.float32, kind="ExternalOutput")

    with tile.TileContext(nc) as tc:
        with (
            tc.tile_pool(name="const", bufs=1) as constp,
            tc.tile_pool(name="state", bufs=1) as statep,
            tc.tile_pool(name="ohp", bufs=3) as ohp,
            tc.tile_pool(name="gates", bufs=3) as gatesp,
            tc.tile_pool(name="work", bufs=3) as workp,
            tc.tile_pool(name="psum", bufs=1, space="PSUM") as psump,
        ):
            eph_sb = constp.tile([128, 2 * 4 * H], dt.float8e4, tag="eph")
            epl_sb = constp.tile([128, 2 * 4 * H], dt.float8e4, tag="epl")
            whh_sb = constp.tile([128, 2 * 4 * H], dt.float8e4, tag="whh")
            # constants off the SP DMA queue so it starts on the step-0
            # one-hot immediately; x-path tables go via the ACT queue (ready
            # first), h-path constants via the Pool SWDGE ring
            nc.scalar.dma_start(out=eph_sb, in_=eph_d[:])
            nc.scalar.dma_start(out=epl_sb, in_=epl_d[:])
            nc.gpsimd.dma_start(out=whh_sb, in_=whh_d[:])

            h8 = statep.tile([128, 2 * C], dt.float8e4, tag="h8")
            cst = statep.tile([128, 2 * C], dt.float16, tag="cst")
            hf = statep.tile([128, 2 * C], dt.float32, tag="hf")
            nc.vector.memset(hf[:], 0.0)

            # step-2 initial state lives in permanent tiles, loaded once, so
            # iterations of the timing loop have no state-reload dependency:
            # step 0 reads these and writes the working tiles.
            c_init = constp.tile([128, 2 * C], dt.float16, tag="c_init")
            h8_init = constp.tile([128, 2 * C], dt.float8e4, tag="h8_init")
            nc.gpsimd.dma_start(out=h8_init[:], in_=h0_d[:])
            nc.gpsimd.dma_start(out=c_init[:], in_=c0_d[:])

            def stat_ap(tbl, m):
                # stationary [K=128, 2 planes, 128] for gate-dim chunk m
                return bass.AP(
                    tensor=tbl.tensor,
                    offset=tbl.offset + m * 128,
                    ap=[tbl.ap[0], [4 * H, 2], [1, 128]],
                )

            def emit_iteration():
                for t in range(WD):
                    At = A[t]
                    if At == 0:
                        break
                    Atn = A[t + 1] if t + 1 < WD else 0

                    oh = ohp.tile([128, 2 * C], dt.float8e4, tag="oh")
                    src = oh_d[t]
                    # at t=0 land the first matmul window's one-hots in a
                    # separate small DMA so the startup chain is short
                    oh_cuts = [0, 256, At] if t == 0 and At > 256 else [0, At]
                    for ci in range(len(oh_cuts) - 1):
                        lo, hi = oh_cuts[ci], oh_cuts[ci + 1]
                        nc.sync.dma_start(
                            out=_pl(oh, C, lo, hi - lo),
                            in_=bass.AP(
                                tensor=src.tensor,
                                offset=src.offset + lo,
                                ap=[src.ap[0], [C, 2], [1, hi - lo]],
                            ),
                        )

                    go_t = gatesp.tile([128, 2 * C], dt.float16, tag="g3",
                                       name="g3")
                    # chunk bounds; smaller leading chunks at t=0 so the
                    # first activation starts early in the startup chain
                    if t == 0:
                        bounds = [0, min(256, At)]
                        while bounds[-1] < At:
                            bounds.append(min(At, bounds[-1] + ch))
                    else:
                        bounds = list(range(0, At, ch)) + [At]
                    for q in range(len(bounds) - 1):
                        qlo = bounds[q]
                        bq = bounds[q + 1] - qlo
                        nw = -(-bq // mmw)

                        pst = {}
                        gts = {}

                        def gref(g, lo, w):
                            gt, base = gts[g]
                            return bass.AP(
                                tensor=gt.tensor,
                                offset=gt.offset + base + lo,
                                ap=[gt.ap[0], [ch, 2], [1, w]],
                            )

                        def sweep(g, tbl, hi, sp):
                            pt, base = pst[g]
                            for msub in range(2):
                                sap = stat_ap(tbl, g * 2 + msub)
                                for wdx in range(nw):
                                    wlo = qlo + wdx * mmw
                                    bw = min(mmw, bq - wdx * mmw)
                                    mv = (
                                        _pl(h8 if t else h8_init, C, wlo, bw)
                                        if tbl is whh_sb
                                        else _pl(oh, C, wlo, bw)
                                    )
                                    off = base + msub * ch + wdx * mmw
                                    # start=True zeroes the full 2KB PSUM bank:
                                    # assert only on bank-aligned windows
                                    st = hi and off % 512 == 0
                                    nc.tensor.matmul(
                                        pt[:, off : off + bw],
                                        sap,
                                        mv,
                                        start=st,
                                        stop=sp,
                                        perf_mode=DR,
                                    )

                        # PSUM: i and f share one tile so a single Sigmoid
                        # instruction covers both (4 planes); g and o separate
                        t_if = psump.tile([128, 4 * ch], dt.float32, tag="Tif",
                                          name="Tif")
                        t_g = psump.tile([128, 2 * ch], dt.float32, tag="Tg",
                                         name="Tg")
                        t_o = psump.tile([128, 2 * ch], dt.float32, tag="To",
                                         name="To")
                        pst = {0: (t_if, 0), 1: (t_if, 2 * ch),
                               2: (t_g, 0), 3: (t_o, 0)}
                        # PE order: (i, f) complete first so the paired
                        # sigma(if) fires early; x-parts precede h-parts
                        for g in (0, 1):
                            sweep(g, eph_sb, True, False)
                            if use_epl:
                                sweep(g, epl_sb, False, False)
                        for g in (0, 1):
                            sweep(g, whh_sb, False, True)
                        for g in (2, 3):
                            sweep(g, eph_sb, True, False)
                            if use_epl:
                                sweep(g, epl_sb, False, False)
                        for g in (2, 3):
                            sweep(g, whh_sb, False, True)

                        gif = gatesp.tile([128, 4 * ch], dt.float16, tag="gif",
                                          name="gif")
                        gts = {0: (gif, 0), 1: (gif, 2 * ch)}
                        if pair_if:
                            # single-chunk steps: sigma(if) in two column
                            # windows so ACT restarts as soon as the first
                            # h-sweep lands
                            sig_cuts = (
                                [0, mmw, bq]
                                if (sigsplit and len(bounds) == 2 and bq > mmw)
                                else [0, bq]
                            )
                            for si in range(len(sig_cuts) - 1):
                                slo, shi = sig_cuts[si], sig_cuts[si + 1]
                                nc.scalar.activation(
                                    bass.AP(tensor=gif.tensor,
                                            offset=gif.offset + slo,
                                            ap=[gif.ap[0], [ch, 4],
                                                [1, shi - slo]]),
                                    bass.AP(tensor=t_if.tensor,
                                            offset=t_if.offset + slo,
                                            ap=[t_if.ap[0], [ch, 4],
                                                [1, shi - slo]]),
                                    AF.Sigmoid,
                                    scale=1.0 / SC,
                                )
                        else:
                            for g, base in ((0, 0), (1, 2 * ch)):
                                nc.scalar.activation(
                                    bass.AP(tensor=gif.tensor,
                                            offset=gif.offset + base,
                                            ap=[gif.ap[0], [ch, 2], [1, bq]]),
                                    bass.AP(tensor=t_if.tensor,
                                            offset=t_if.offset + base,
                                            ap=[t_if.ap[0], [ch, 2], [1, bq]]),
                                    AF.Sigmoid,
                                    scale=1.0 / SC,
                                )
                        fc = workp.tile([128, 2 * ch], dt.float16, tag="fc")
                        nc.vector.tensor_mul(
                            _pl(fc, ch, 0, bq),
                            gref(1, 0, bq),
                            _pl(cst if t else c_init, C, qlo, bq),
                        )
                        gg = gatesp.tile([128, 2 * ch], dt.float16, tag="g2",
                                         name="g2")
                        gts[2] = (gg, 0)
                        nc.scalar.activation(
                            _pl(gg, ch, 0, bq), _pl(t_g, ch, 0, bq),
                            AF.Tanh, scale=1.0 / SC,
                        )
                        ig = workp.tile([128, 2 * ch], dt.float16, tag="ig")
                        nc.vector.tensor_mul(
                            _pl(ig, ch, 0, bq), gref(0, 0, bq), gref(2, 0, bq)
                        )
                        nc.vector.tensor_add(
                            _pl(cst, C, qlo, bq),
                            _pl(ig, ch, 0, bq),
                            _pl(fc, ch, 0, bq),
                        )
                        nc.scalar.activation(
                            _pl(go_t, C, qlo, bq), _pl(t_o, ch, 0, bq),
                            AF.Sigmoid, scale=1.0 / SC,
                        )

                        th = workp.tile([128, 2 * C], dt.float16, tag="th")
                        hb = max(0, min(Atn - qlo, bq))
                        nc.scalar.activation(
                            _pl(th, C, qlo, bq), _pl(cst, C, qlo, bq), AF.Tanh
                        )
                        if hb > 0:
                            h8_eng = nc.gpsimd if h8_pool else nc.vector
                            h8_eng.tensor_mul(
                                _pl(h8, C, qlo, hb), _pl(go_t, C, qlo, hb),
                                _pl(th, C, qlo, hb),
                            )
                        if bq - hb > 0:
                            # finishers are off the recurrence critical path:
                            # compute on Pool to relieve DVE
                            (nc.gpsimd if hf_pool else nc.vector).tensor_mul(
                                _pl(hf, C, qlo + hb, bq - hb),
                                _pl(go_t, C, qlo + hb, bq - hb),
                                _pl(th, C, qlo + hb, bq - hb),
                            )

                    # stream out the columns that finished at this step
                    # (single DMA covering both 128-row planes)
                    fin_lo, fin_hi = Atn, At
                    if fin_hi > fin_lo:
                        base = out_d[0]
                        nc.sync.dma_start(
                            out=bass.AP(
                                tensor=base.tensor,
                                offset=base.offset + fin_lo,
                                ap=[base.ap[0], [128 * C, 2],
                                    [1, fin_hi - fin_lo]],
                            ),
                            in_=bass.AP(
                                tensor=hf.tensor,
                                offset=hf.offset + fin_lo,
                                ap=[hf.ap[0], [C, 2], [1, fin_hi - fin_lo]],
                            ),
                        )

            if reps == 1:
                emit_iteration()
            else:
                with tc.For_i(0, reps, 1):
                    emit_iteration()

    nc.compile()
    _PROGRAM_CACHE[key] = nc
    return nc


def _host_tables(embedding, W_ih, W_hh, b_ih, b_hh):
    """Input-independent weight tables: embproj + first/second step states."""
    keyb = hashlib.sha1()
    for a in (embedding, W_ih, W_hh, b_ih, b_hh):
        keyb.update(np.ascontiguousarray(a).tobytes())
    key = keyb.hexdigest()
    if key in _TABLE_CACHE:
        return _TABLE_CACHE[key]

    emb = np.asarray(embedding, np.float64)
    Wih = np.asarray(W_ih, np.float64)
    Whh = np.asarray(W_hh, np.float64)
    bih = np.asarray(b_ih, np.float64)
    bhh = np.asarray(b_hh, np.float64)

    ep = (emb @ Wih.T + bih + bhh).astype(np.float32)  # [V, 4H], bias folded
    t1 = np.clip(ep * SC, -240, 240).astype(E4)
    t2 = np.clip(ep * SC - t1.astype(np.float32), -240, 240).astype(E4)

    def dr_layout(tbl):  # [V, 4H] -> [128, 2*4H], k=(p, plane): v = plane*128+p
        return np.ascontiguousarray(
            tbl.reshape(2, 128, 4 * H).transpose(1, 0, 2).reshape(128, 2 * 4 * H)
        )

    whh_q = np.clip(np.asarray(W_hh, np.float32).T * SC, -240, 240).astype(E4)

    def sig(z):
        return 1.0 / (1.0 + np.exp(-z))

    # step-0 tables over the vocab (h=0, c=0): state after consuming char v
    g0 = ep.astype(np.float64)  # [V, 4H]
    i0, f0, gg0, o0 = (g0[:, :H], g0[:, H : 2 * H], g0[:, 2 * H : 3 * H], g0[:, 3 * H :])
    c1 = sig(i0) * np.tanh(gg0)  # [V, H]
    h1 = sig(o0) * np.tanh(c1)  # [V, H]

    # step-1 pair tables: state after chars (a, b), computed in f32 blocks
    hh1 = (h1 @ Whh.T).astype(np.float32)  # [V, 4H]
    ep32 = ep  # [V, 4H] f32
    c1f = c1.astype(np.float32)
    c2 = np.empty((V, V, H), np.float32)
    h2 = np.empty((V, V, H), np.float32)
    blk = 32
    for a0 in range(0, V, blk):
        a1 = min(V, a0 + blk)
        gates = hh1[a0:a1, None, :] + ep32[None, :, :]  # [blk, V, 4H]
        i = sig(gates[..., :H])
        f = sig(gates[..., H : 2 * H])
        g = np.tanh(gates[..., 2 * H : 3 * H])
        o = sig(gates[..., 3 * H :])
        cc = f * c1f[a0:a1, None, :] + i * g
        c2[a0:a1] = cc
        h2[a0:a1] = o * np.tanh(cc)

    tables = {
        "eph": dr_layout(t1),
        "epl": dr_layout(t2),
        "whh": dr_layout(whh_q),
        "c1": c1.astype(np.float32),
        "h1": h1.astype(np.float32),
        "c2": c2,
        "h2": h2,
    }
    _TABLE_CACHE.clear()
    _TABLE_CACHE[key] = tables
    return tables


def _prepare(char_input, embedding, W_ih, W_hh, b_ih, b_hh, round_to=2):
    ci = np.asarray(char_input)
    chars = ci.reshape(-1, W).astype(np.int64)
    lens = (chars != 0).sum(-1)

    tb = _host_tables(embedding, W_ih, W_hh, b_ih, b_hh)

    # host-resolved short words
    out_host = np.zeros((B * S, H), np.float32)
    sel1 = lens == 1
    out_host[sel1] = tb["h1"][chars[sel1, 0]]
    sel2 = lens == 2
    out_host[sel2] = tb["h2"][chars[sel2, 0], chars[sel2, 1]]

    # device words: len >= 3, device len' = len - 2
    dev = np.nonzero(lens >= 3)[0]
    dlens = lens[dev] - HOST_STEPS
    colsL, C, A = _plan(dlens, round_to)
    colmap = _assign(dlens, dev, colsL, C)

    tgrid = np.broadcast_to(np.arange(WD)[:, None], (WD, C))
    cgrid = np.broadcast_to(np.arange(C)[None, :], (WD, C))
    in_maps = []
    for k in range(NCORES):
        mask = colmap[k] >= 0
        wid = colmap[k]
        # chars at positions t=2.. for each column ([WD, C])
        cc = np.zeros((C, WD), np.int64)
        cc[mask] = chars[wid[mask], HOST_STEPS:]
        v = cc.T  # [WD, C]
        oh = np.zeros((WD, 128, 2 * C), np.uint8)
        oh[tgrid, v % 128, (v // 128) * C + cgrid] = 0x38  # 1.0 in e4m3
        # initial state from pair tables: [C, H] -> [128, 2C] (plane*128+p)
        c2w = np.zeros((C, H), np.float32)
        h2w = np.zeros((C, H), np.float32)
        c2w[mask] = tb["c2"][chars[wid[mask], 0], chars[wid[mask], 1]]
        h2w[mask] = tb["h2"][chars[wid[mask], 0], chars[wid[mask], 1]]

        def st_layout(x, dtype):  # [C, H] -> [128, 2C]
            return np.ascontiguousarray(
                x.reshape(C, 2, 128).transpose(2, 1, 0).reshape(128, 2 * C)
            ).astype(dtype)

        in_maps.append(
            {
                "oh": oh.view(E4),
                "eph": tb["eph"],
                "epl": tb["epl"],
                "whh": tb["whh"],
                "c0": st_layout(c2w, np.float16),
                "h0": st_layout(h2w, E4),
            }
        )
    return colmap, in_maps, C, A, out_host


def _gather_output(results, colmap, out_host):
    out_flat = out_host.copy()
    for k in range(NCORES):
        o = results[k]["out"].astype(np.float32)  # [2, 128, C]
        h_core = o.reshape(H, o.shape[-1])
        mask = colmap[k] >= 0
        out_flat[colmap[k][mask]] = h_core[:, mask].T
    return out_flat.reshape(B, S, H)


def kernel(char_input, embedding, W_ih, W_hh, b_ih, b_hh):
    colmap, in_maps, C, A, out_host = _prepare(
        char_input, embedding, W_ih, W_hh, b_ih, b_hh, round_to=2
    )
    nc = _build_program(C, A)
    res = run_bass_kernel_spmd(nc, in_maps, core_ids=list(range(NCORES)))
    return _gather_output(res.results, colmap, out_host)


# revision 32
# speedup vs baseline: 1.0651x; 1.0651x over previous
"""Character-LSTM Trainium2 kernel V3 (8 NeuronCores, SPMD data-parallel).

Strategy
--------
Words sorted by descending length are dealt round-robin across 8 cores so the
per-step active-column count A[t] is core-uniform; words live as columns of
transposed state tiles.

The first two LSTM steps are resolved on the host from input-independent
weight tables: step 0's state (c1, h1) is a function of the first char only
(V-entry table), and step 1's state (c2, h2) a function of the first char
pair (V^2-entry table). Words of length <= 2 never reach the device; longer
words start on-device at t=2 with (c2, h2) DMA'd in as initial state (c2 in
f16, h2 in fp8). This removes the two widest recurrence steps (~15% of
column-steps) from the device program, whose throughput is bound by the
Activation engine (10 table-lookup elems per column-step).

Per step, gates g[4H x cols] are computed on the PE: the x-part multiplies a
host-built fp8 one-hot of the char against a bf16 embproj table
(emb@W_ih.T + bias) in two K=128 passes (FWL stays enabled), and the h-part
uses fp8 W_hh x fp8 h DoubleRow matmuls contracting K=256 in one pass. The
i/f sigmoids run as ONE ACT instruction over an adjacent PSUM pair (real
per-instruction ACT cost is large, so instruction count matters more than
the simulator suggests); tanh(g), sigma(o) and tanh(c) follow per chunk with
the x16 table scaling folded into the ACT scale. The cell update runs on DVE
in f16; h is stored fp8 for the next step's matmuls, and in fp32 on Pool for
columns finishing at this step. PE sweeps that do not depend on h are
emitted before the h sweeps to keep the recurrence critical path short, and
finished columns stream to HBM per step.
"""

import sys

if "/opt/trn_rl_repo" not in sys.path:
    sys.path.insert(0, "/opt/trn_rl_repo")

import hashlib

import numpy as np
import ml_dtypes

import concourse.bass as bass
import concourse.tile as tile
from concourse import bacc, mybir
from concourse.bass_utils import run_bass_kernel_spmd

E4 = ml_dtypes.float8_e4m3fn
NCORES = 8
B, S, W, E, H, V = 64, 256, 24, 128, 256, 256
HOST_STEPS = 2  # steps resolved on host via vocab/pair tables
WD = W - HOST_STEPS  # device steps
GATE_FUNCS = ["Sigmoid", "Sigmoid", "Tanh", "Sigmoid"]  # i, f, g, o
GORDER = (0, 2, 1, 3)  # emission order: i, g, f, o
SC = 16.0  # fp8 table scale; ACT applies 1/SC
CH = 512  # column chunk (PSUM: 4 gate tags x [128, 2*CH] fp32 = 8 banks)
MMW = 512  # DoubleRow matmul window (one PSUM bank of output)

_PROGRAM_CACHE: dict = {}
_TABLE_CACHE: dict = {}


def _plan(lens: np.ndarray, round_to: int = 1):
    """Column counts per device step, uniform across cores.

    lens here are DEVICE lens (len-2, in [1, WD]); words with len<=2 are
    excluded before calling.
    """
    wL = np.bincount(lens, minlength=WD + 1)
    colsL = np.zeros(WD + 1, np.int64)
    cum = 0
    for L in range(WD, 0, -1):
        need = -(-int(wL[L]) // NCORES)
        newcum = -(-(cum + need) // round_to) * round_to
        colsL[L] = newcum - cum
        cum = newcum
    C = max(cum, 16)
    A = [int(colsL[t + 1 :].sum()) for t in range(WD)]
    return colsL, C, A


def _assign(lens, ids_all, colsL, C):
    """Deal device words into (core, column) slots, longest first."""
    order = np.argsort(-lens, kind="stable")
    wL = np.bincount(lens, minlength=WD + 1)
    colmap = np.full((NCORES, C), -1, np.int64)
    pos = 0
    s = 0
    for L in range(WD, 0, -1):
        cnt = int(wL[L])
        if cnt:
            sel = order[pos : pos + cnt]
            pos += cnt
            k = np.arange(cnt) % NCORES
            j = s + np.arange(cnt) // NCORES
            colmap[k, j] = ids_all[sel]
        s += int(colsL[L])
    return colmap


def _pl(t, stride, lo, w):
    """AP over tile t: [128 part, 2 planes (stride), w cols] at col offset lo."""
    return bass.AP(
        tensor=t.tensor, offset=t.offset + lo, ap=[t.ap[0], [stride, 2], [1, w]]
    )


def _build_program(C: int, A: list[int], reps: int = 1, ch: int = CH, mmw: int = MMW,
                   hsplit: int = 1, h8_pool: bool = False, pair_if: bool = True,
                   h8win: bool = False, sigsplit: bool = False,
                   tansplit: bool = False, use_epl: bool = True,
                   hf_pool: bool = True, xbf16: bool = True):
    key = (C, tuple(A), reps, ch, mmw, hsplit, h8_pool, pair_if, h8win, sigsplit,
           tansplit, use_epl, hf_pool, xbf16)
    if key in _PROGRAM_CACHE:
        return _PROGRAM_CACHE[key]

    dt = mybir.dt
    AF = mybir.ActivationFunctionType
    DR = mybir.MatmulPerfMode.DoubleRow
    nc = bacc.Bacc("TRN2", target_bir_lowering=False, debug=False, num_devices=NCORES)

    oh_d = nc.dram_tensor("oh", [WD, 128, 2 * C], dt.float8e4, kind="ExternalInput")
    eph_d = nc.dram_tensor("eph", [128, 2 * 4 * H], dt.float8e4, kind="ExternalInput")
    epb_d = nc.dram_tensor("epb", [128, 2 * 4 * H], dt.bfloat16, kind="ExternalInput")
    epl_d = nc.dram_tensor("epl", [128, 2 * 4 * H], dt.float8e4, kind="ExternalInput")
    whh_d = nc.dram_tensor("whh", [128, 2 * 4 * H], dt.float8e4, kind="ExternalInput")
    c0_d = nc.dram_tensor("c0", [128, 2 * C], dt.float16, kind="ExternalInput")
    h0_d = nc.dram_tensor("h0", [128, 2 * C], dt.float8e4, kind="ExternalInput")
    out_d = nc.dram_tensor("out", [2, 128, C], dt.float32, kind="ExternalOutput")

    with tile.TileContext(nc) as tc:
        with (
            tc.tile_pool(name="const", bufs=1) as constp,
            tc.tile_pool(name="state", bufs=1) as statep,
            tc.tile_pool(name="ohp", bufs=3) as ohp,
            tc.tile_pool(name="gates", bufs=3) as gatesp,
            tc.tile_pool(name="work", bufs=3) as workp,
            tc.tile_pool(name="psum", bufs=1, space="PSUM") as psump,
        ):
            eph_sb = constp.tile([128, 2 * 4 * H], dt.float8e4, tag="eph")
            epl_sb = constp.tile([128, 2 * 4 * H], dt.float8e4, tag="epl")
            epb_sb = constp.tile([128, 2 * 4 * H], dt.bfloat16, tag="epb")
            whh_sb = constp.tile([128, 2 * 4 * H], dt.float8e4, tag="whh")
            # constants off the SP DMA queue so it starts on the step-0
            # one-hot immediately; x-path tables go via the ACT queue (ready
            # first), h-path constants via the Pool SWDGE ring
            if xbf16:
                nc.scalar.dma_start(out=epb_sb, in_=epb_d[:])
            else:
                nc.scalar.dma_start(out=eph_sb, in_=eph_d[:])
                nc.scalar.dma_start(out=epl_sb, in_=epl_d[:])
            nc.gpsimd.dma_start(out=whh_sb, in_=whh_d[:])

            h8 = statep.tile([128, 2 * C], dt.float8e4, tag="h8")
            cst = statep.tile([128, 2 * C], dt.float16, tag="cst")
            hf = statep.tile([128, 2 * C], dt.float32, tag="hf")
            nc.vector.memset(hf[:], 0.0)

            # step-2 initial state lives in permanent tiles, loaded once, so
            # iterations of the timing loop have no state-reload dependency:
            # step 0 reads these and writes the working tiles.
            c_init = constp.tile([128, 2 * C], dt.float16, tag="c_init")
            h8_init = constp.tile([128, 2 * C], dt.float8e4, tag="h8_init")
            nc.gpsimd.dma_start(out=h8_init[:], in_=h0_d[:])
            nc.gpsimd.dma_start(out=c_init[:], in_=c0_d[:])

            def stat_ap(tbl, m):
                # stationary [K=128, 2 planes, 128] for gate-dim chunk m
                return bass.AP(
                    tensor=tbl.tensor,
                    offset=tbl.offset + m * 128,
                    ap=[tbl.ap[0], [4 * H, 2], [1, 128]],
                )

            def emit_iteration():
                for t in range(WD):
                    At = A[t]
                    if At == 0:
                        break
                    Atn = A[t + 1] if t + 1 < WD else 0

                    oh = ohp.tile([128, 2 * C], dt.float8e4, tag="oh")
                    src = oh_d[t]
                    # at t=0 land the first matmul window's one-hots in a
                    # separate small DMA so the startup chain is short
                    oh_cuts = [0, 256, At] if t == 0 and At > 256 else [0, At]
                    for ci in range(len(oh_cuts) - 1):
                        lo, hi = oh_cuts[ci], oh_cuts[ci + 1]
                        nc.sync.dma_start(
                            out=_pl(oh, C, lo, hi - lo),
                            in_=bass.AP(
                                tensor=src.tensor,
                                offset=src.offset + lo,
                                ap=[src.ap[0], [C, 2], [1, hi - lo]],
                            ),
                        )

                    go_t = gatesp.tile([128, 2 * C], dt.float16, tag="g3",
                                       name="g3")
                    # chunk bounds; smaller leading chunks at t=0 so the
                    # first activation starts early in the startup chain
                    if t == 0:
                        bounds = [0, min(256, At)]
                        while bounds[-1] < At:
                            bounds.append(min(At, bounds[-1] + ch))
                    else:
                        bounds = list(range(0, At, ch)) + [At]
                    for q in range(len(bounds) - 1):
                        qlo = bounds[q]
                        bq = bounds[q + 1] - qlo
                        nw = -(-bq // mmw)

                        pst = {}
                        gts = {}

                        def gref(g, lo, w):
                            gt, base = gts[g]
                            return bass.AP(
                                tensor=gt.tensor,
                                offset=gt.offset + base + lo,
                                ap=[gt.ap[0], [ch, 2], [1, w]],
                            )

                        def sweep(g, tbl, hi, sp):
                            pt, base = pst[g]
                            for msub in range(2):
                                sap = stat_ap(tbl, g * 2 + msub)
                                for wdx in range(nw):
                                    wlo = qlo + wdx * mmw
                                    bw = min(mmw, bq - wdx * mmw)
                                    mv = (
                                        _pl(h8 if t else h8_init, C, wlo, bw)
                                        if tbl is whh_sb
                                        else _pl(oh, C, wlo, bw)
                                    )
                                    off = base + msub * ch + wdx * mmw
                                    # start=True zeroes the full 2KB PSUM bank:
                                    # assert only on bank-aligned windows
                                    st = hi and off % 512 == 0
                                    nc.tensor.matmul(
                                        pt[:, off : off + bw],
                                        sap,
                                        mv,
                                        start=st,
                                        stop=sp,
                                        perf_mode=DR,
                                    )

                        def sweep_x16(g, hi, sp):
                            # x-part with a single bf16 table: two K=128
                            # passes per msub (no DoubleRow, FWL stays on)
                            pt, base = pst[g]
                            for msub in range(2):
                                m = g * 2 + msub
                                for kt in range(2):
                                    sap = bass.AP(
                                        tensor=epb_sb.tensor,
                                        offset=epb_sb.offset + kt * 4 * H
                                        + m * 128,
                                        ap=[epb_sb.ap[0], [1, 128]],
                                    )
                                    for wdx in range(nw):
                                        wlo = qlo + wdx * mmw
                                        bw = min(mmw, bq - wdx * mmw)
                                        mv = bass.AP(
                                            tensor=oh.tensor,
                                            offset=oh.offset + kt * C + wlo,
                                            ap=[oh.ap[0], [1, bw]],
                                        )
                                        off = base + msub * ch + wdx * mmw
                                        st = hi and kt == 0 and off % 512 == 0
                                        nc.tensor.matmul(
                                            pt[:, off : off + bw],
                                            sap,
                                            mv,
                                            start=st,
                                            stop=sp,
                                        )

                        # PSUM: i and f share one tile so a single Sigmoid
                        # instruction covers both (4 planes); g and o separate
                        t_if = psump.tile([128, 4 * ch], dt.float32, tag="Tif",
                                          name="Tif")
                        t_g = psump.tile([128, 2 * ch], dt.float32, tag="Tg",
                                         name="Tg")
                        t_o = psump.tile([128, 2 * ch], dt.float32, tag="To",
                                         name="To")
                        pst = {0: (t_if, 0), 1: (t_if, 2 * ch),
                               2: (t_g, 0), 3: (t_o, 0)}
                        # PE order: (i, f) complete first so the paired
                        # sigma(if) fires early; x-parts precede h-parts
                        def xsweep(g, hi):
                            if xbf16:
                                sweep_x16(g, hi, False)
                            else:
                                sweep(g, eph_sb, hi, False)
                                if use_epl:
                                    sweep(g, epl_sb, False, False)

                        for g in (0, 1):
                            xsweep(g, True)
                        for g in (0, 1):
                            sweep(g, whh_sb, False, True)
                        for g in (2, 3):
                            xsweep(g, True)
                        for g in (2, 3):
                            sweep(g, whh_sb, False, True)

                        gif = gatesp.tile([128, 4 * ch], dt.float16, tag="gif",
                                          name="gif")
                        gts = {0: (gif, 0), 1: (gif, 2 * ch)}
                        if pair_if:
                            # single-chunk steps: sigma(if) in two column
                            # windows so ACT restarts as soon as the first
                            # h-sweep lands
                            sig_cuts = (
                                [0, mmw, bq]
                                if (sigsplit and len(bounds) == 2 and bq > mmw)
                                else [0, bq]
                            )
                            for si in range(len(sig_cuts) - 1):
                                slo, shi = sig_cuts[si], sig_cuts[si + 1]
                                nc.scalar.activation(
                                    bass.AP(tensor=gif.tensor,
                                            offset=gif.offset + slo,
                                            ap=[gif.ap[0], [ch, 4],
                                                [1, shi - slo]]),
                                    bass.AP(tensor=t_if.tensor,
                                            offset=t_if.offset + slo,
                                            ap=[t_if.ap[0], [ch, 4],
                                                [1, shi - slo]]),
                                    AF.Sigmoid,
                                    scale=1.0 / SC,
                                )
                        else:
                            for g, base in ((0, 0), (1, 2 * ch)):
                                nc.scalar.activation(
                                    bass.AP(tensor=gif.tensor,
                                            offset=gif.offset + base,
                                            ap=[gif.ap[0], [ch, 2], [1, bq]]),
                                    bass.AP(tensor=t_if.tensor,
                                            offset=t_if.offset + base,
                                            ap=[t_if.ap[0], [ch, 2], [1, bq]]),
                                    AF.Sigmoid,
                                    scale=1.0 / SC,
                                )
                        fc = workp.tile([128, 2 * ch], dt.float16, tag="fc")
                        nc.vector.tensor_mul(
                            _pl(fc, ch, 0, bq),
                            gref(1, 0, bq),
                            _pl(cst if t else c_init, C, qlo, bq),
                        )
                        gg = gatesp.tile([128, 2 * ch], dt.float16, tag="g2",
                                         name="g2")
                        gts[2] = (gg, 0)
                        nc.scalar.activation(
                            _pl(gg, ch, 0, bq), _pl(t_g, ch, 0, bq),
                            AF.Tanh, scale=1.0 / SC,
                        )
                        ig = workp.tile([128, 2 * ch], dt.float16, tag="ig")
                        nc.vector.tensor_mul(
                            _pl(ig, ch, 0, bq), gref(0, 0, bq), gref(2, 0, bq)
                        )
                        nc.vector.tensor_add(
                            _pl(cst, C, qlo, bq),
                            _pl(ig, ch, 0, bq),
                            _pl(fc, ch, 0, bq),
                        )
                        nc.scalar.activation(
                            _pl(go_t, C, qlo, bq), _pl(t_o, ch, 0, bq),
                            AF.Sigmoid, scale=1.0 / SC,
                        )

                        th = workp.tile([128, 2 * C], dt.float16, tag="th")
                        hb = max(0, min(Atn - qlo, bq))
                        nc.scalar.activation(
                            _pl(th, C, qlo, bq), _pl(cst, C, qlo, bq), AF.Tanh
                        )
                        if hb > 0:
                            h8_eng = nc.gpsimd if h8_pool else nc.vector
                            h8_eng.tensor_mul(
                                _pl(h8, C, qlo, hb), _pl(go_t, C, qlo, hb),
                                _pl(th, C, qlo, hb),
                            )
                        if bq - hb > 0:
                            # finishers are off the recurrence critical path:
                            # compute on Pool to relieve DVE
                            (nc.gpsimd if hf_pool else nc.vector).tensor_mul(
                                _pl(hf, C, qlo + hb, bq - hb),
                                _pl(go_t, C, qlo + hb, bq - hb),
                                _pl(th, C, qlo + hb, bq - hb),
                            )

                    # stream out the columns that finished at this step
                    # (single DMA covering both 128-row planes)
                    fin_lo, fin_hi = Atn, At
                    if fin_hi > fin_lo:
                        base = out_d[0]
                        nc.sync.dma_start(
                            out=bass.AP(
                                tensor=base.tensor,
                                offset=base.offset + fin_lo,
                                ap=[base.ap[0], [128 * C, 2],
                                    [1, fin_hi - fin_lo]],
                            ),
                            in_=bass.AP(
                                tensor=hf.tensor,
                                offset=hf.offset + fin_lo,
                                ap=[hf.ap[0], [C, 2], [1, fin_hi - fin_lo]],
                            ),
                        )

            if reps == 1:
                emit_iteration()
            else:
                with tc.For_i(0, reps, 1):
                    emit_iteration()

    nc.compile()
    _PROGRAM_CACHE[key] = nc
    return nc


def _host_tables(embedding, W_ih, W_hh, b_ih, b_hh):
    """Input-independent weight tables: embproj + first/second step states."""
    keyb = hashlib.sha1()
    for a in (embedding, W_ih, W_hh, b_ih, b_hh):
        keyb.update(np.ascontiguousarray(a).tobytes())
    key = keyb.hexdigest()
    if key in _TABLE_CACHE:
        return _TABLE_CACHE[key]

    emb = np.asarray(embedding, np.float64)
    Wih = np.asarray(W_ih, np.float64)
    Whh = np.asarray(W_hh, np.float64)
    bih = np.asarray(b_ih, np.float64)
    bhh = np.asarray(b_hh, np.float64)

    ep = (emb @ Wih.T + bih + bhh).astype(np.float32)  # [V, 4H], bias folded
    t1 = np.clip(ep * SC, -240, 240).astype(E4)
    t2 = np.clip(ep * SC - t1.astype(np.float32), -240, 240).astype(E4)

    def dr_layout(tbl):  # [V, 4H] -> [128, 2*4H], k=(p, plane): v = plane*128+p
        return np.ascontiguousarray(
            tbl.reshape(2, 128, 4 * H).transpose(1, 0, 2).reshape(128, 2 * 4 * H)
        )

    whh_q = np.clip(np.asarray(W_hh, np.float32).T * SC, -240, 240).astype(E4)

    def sig(z):
        return 1.0 / (1.0 + np.exp(-z))

    # step-0 tables over the vocab (h=0, c=0): state after consuming char v
    g0 = ep.astype(np.float64)  # [V, 4H]
    i0, f0, gg0, o0 = (g0[:, :H], g0[:, H : 2 * H], g0[:, 2 * H : 3 * H], g0[:, 3 * H :])
    c1 = sig(i0) * np.tanh(gg0)  # [V, H]
    h1 = sig(o0) * np.tanh(c1)  # [V, H]

    # step-1 pair tables: state after chars (a, b), computed in f32 blocks
    hh1 = (h1 @ Whh.T).astype(np.float32)  # [V, 4H]
    ep32 = ep  # [V, 4H] f32
    c1f = c1.astype(np.float32)
    c2 = np.empty((V, V, H), np.float32)
    h2 = np.empty((V, V, H), np.float32)
    blk = 32
    for a0 in range(0, V, blk):
        a1 = min(V, a0 + blk)
        gates = hh1[a0:a1, None, :] + ep32[None, :, :]  # [blk, V, 4H]
        i = sig(gates[..., :H])
        f = sig(gates[..., H : 2 * H])
        g = np.tanh(gates[..., 2 * H : 3 * H])
        o = sig(gates[..., 3 * H :])
        cc = f * c1f[a0:a1, None, :] + i * g
        c2[a0:a1] = cc
        h2[a0:a1] = o * np.tanh(cc)

    tables = {
        "eph": dr_layout(t1),
        "epl": dr_layout(t2),
        "epb": dr_layout((ep * SC).astype(ml_dtypes.bfloat16)),
        "whh": dr_layout(whh_q),
        "c1": c1.astype(np.float32),
        "h1": h1.astype(np.float32),
        "c2": c2,
        "h2": h2,
    }
    _TABLE_CACHE.clear()
    _TABLE_CACHE[key] = tables
    return tables


def _prepare(char_input, embedding, W_ih, W_hh, b_ih, b_hh, round_to=2):
    ci = np.asarray(char_input)
    chars = ci.reshape(-1, W).astype(np.int64)
    lens = (chars != 0).sum(-1)

    tb = _host_tables(embedding, W_ih, W_hh, b_ih, b_hh)

    # host-resolved short words
    out_host = np.zeros((B * S, H), np.float32)
    sel1 = lens == 1
    out_host[sel1] = tb["h1"][chars[sel1, 0]]
    sel2 = lens == 2
    out_host[sel2] = tb["h2"][chars[sel2, 0], chars[sel2, 1]]

    # device words: len >= 3, device len' = len - 2
    dev = np.nonzero(lens >= 3)[0]
    dlens = lens[dev] - HOST_STEPS
    colsL, C, A = _plan(dlens, round_to)
    colmap = _assign(dlens, dev, colsL, C)

    tgrid = np.broadcast_to(np.arange(WD)[:, None], (WD, C))
    cgrid = np.broadcast_to(np.arange(C)[None, :], (WD, C))
    in_maps = []
    for k in range(NCORES):
        mask = colmap[k] >= 0
        wid = colmap[k]
        # chars at positions t=2.. for each column ([WD, C])
        cc = np.zeros((C, WD), np.int64)
        cc[mask] = chars[wid[mask], HOST_STEPS:]
        v = cc.T  # [WD, C]
        oh = np.zeros((WD, 128, 2 * C), np.uint8)
        oh[tgrid, v % 128, (v // 128) * C + cgrid] = 0x38  # 1.0 in e4m3
        # initial state from pair tables: [C, H] -> [128, 2C] (plane*128+p)
        c2w = np.zeros((C, H), np.float32)
        h2w = np.zeros((C, H), np.float32)
        c2w[mask] = tb["c2"][chars[wid[mask], 0], chars[wid[mask], 1]]
        h2w[mask] = tb["h2"][chars[wid[mask], 0], chars[wid[mask], 1]]

        def st_layout(x, dtype):  # [C, H] -> [128, 2C]
            return np.ascontiguousarray(
                x.reshape(C, 2, 128).transpose(2, 1, 0).reshape(128, 2 * C)
            ).astype(dtype)

        in_maps.append(
            {
                "oh": oh.view(E4),
                "eph": tb["eph"],
                "epl": tb["epl"],
                "epb": tb["epb"],
                "whh": tb["whh"],
                "c0": st_layout(c2w, np.float16),
                "h0": st_layout(h2w, E4),
            }
        )
    return colmap, in_maps, C, A, out_host


def _gather_output(results, colmap, out_host):
    out_flat = out_host.copy()
    for k in range(NCORES):
        o = results[k]["out"].astype(np.float32)  # [2, 128, C]
        h_core = o.reshape(H, o.shape[-1])
        mask = colmap[k] >= 0
        out_flat[colmap[k][mask]] = h_core[:, mask].T
    return out_flat.reshape(B, S, H)


def kernel(char_input, embedding, W_ih, W_hh, b_ih, b_hh):
    colmap, in_maps, C, A, out_host = _prepare(
        char_input, embedding, W_ih, W_hh, b_ih, b_hh, round_to=2
    )
    nc = _build_program(C, A)
    res = run_bass_kernel_spmd(nc, in_maps, core_ids=list(range(NCORES)))
    return _gather_output(res.results, colmap, out_host)
